# revision 12
# baseline (speedup 1.0000x reference)
"""Trainium2 Bass kernel for nn_AdvancedRegressionModel (20-qubit circuit regression).

Math: the reference circuit is out_b = sum_j g_j |(A psi_b)_j|^2 + b0 where
A = kron of 20 fused 2x2 gates (RY_k @ RX_k per wire) and g absorbs the CNOT
chain (a basis permutation), the <Z_i> measurements, and the linear head via
prefix-sign weights.  Split wires into blocks a(0-5), m(6-12), z(13-19);
g(a,m,z) = ga(a) + sA(a)*gm(m) + sA(a)*sM(m)*gz(z).

Device pipeline per batch row (4 rows/core, batch-sharded over 8 cores):
  load  [z:128, (a,m):8192]                                        (DMA)
  P1    contract z (stat-data matmuls, real -> complex)            (PE)
  P2    contract m (stat-data matmuls per z'-pair tile)            (PE)
  P3    apply kron(Ua, I2) on p=(a,z2) partitions (normal matmuls) (PE)
  SQ    squares of re/im (ScalarE, PSUM->SBUF)                     (ACT)
  P4    reduce-matmul with 4 sign-weight columns -> F[4, 8192]     (PE)
  host  finish: out_r = F0.1 + F1.gm + F2.sM*gz(t) + F3.sM*gz(t+64) + b

f32r matmuls carry at most ONE sync-wait, so cross-engine dependency
frontiers are funneled through tiny "touch" matmuls (one producer-proc each)
and PSUM drains alternate whole-group between DVE/ACT so slot-recycle
dependencies stay single-proc.
"""

import numpy as np
from contextlib import ExitStack

import concourse.mybir as mybir
from concourse import bacc, bass, tile
from concourse.bass_utils import run_bass_kernel_spmd
from concourse.tile_rust import add_dep_helper

NW = 20
DIM = 2 ** NW
BATCH = 32
NCORES = 8
RPC = BATCH // NCORES  # rows per core

F32 = mybir.dt.float32
F32R = mybir.dt.float32r


# ----------------------------------------------------------------- host math
def _gates(params):
    A = []
    for k in range(NW):
        c, s = np.cos(params[k] / 2), np.sin(params[k] / 2)
        RX = np.array([[c, -1j * s], [-1j * s, c]], dtype=np.complex128)
        c2, s2 = np.cos(params[k + NW] / 2), np.sin(params[k + NW] / 2)
        RY = np.array([[c2, -s2], [s2, c2]], dtype=np.complex128)
        A.append(RY @ RX)
    return A


def _kron_list(lst):
    out = lst[0]
    for x in lst[1:]:
        out = np.kron(out, x)
    return out


def _prefix_weights(Wv, wire_lo, wire_hi):
    n = wire_hi - wire_lo + 1
    v = np.arange(2 ** n)
    signs = np.stack([1 - 2 * ((v >> (n - 1 - k)) & 1) for k in range(n)])
    pref = np.cumprod(signs, axis=0)
    g = sum(Wv[wire_lo + i] * pref[i] for i in range(n))
    return g.astype(np.float64), pref[n - 1].astype(np.float64)


def _host_precompute(params, W):
    params = np.asarray(params, dtype=np.float64)
    Wv = np.asarray(W, dtype=np.float64).reshape(-1)
    A = _gates(params)
    Ua = _kron_list(A[0:6])      # 64x64
    Um = _kron_list(A[6:13])     # 128x128
    Uz = _kron_list(A[13:20])    # 128x128
    G3 = np.kron(Ua, np.eye(2))  # 128x128 acting on p = 2a + h

    G1 = np.concatenate([Uz.real.T, Uz.imag.T], axis=1)
    G2a = np.concatenate([Um.real.T, Um.imag.T], axis=1)
    G2b = np.concatenate([-Um.imag.T, Um.real.T], axis=1)
    G3S = np.concatenate([G3.real.T, G3.imag.T, -G3.imag.T], axis=1)

    ga, sA = _prefix_weights(Wv, 0, 5)    # [64]
    gm, sM = _prefix_weights(Wv, 6, 12)   # [128]
    gz, _ = _prefix_weights(Wv, 13, 19)  # [128]

    p = np.arange(128)
    av, z2 = p >> 1, p & 1
    R = np.stack([ga[av], sA[av], sA[av] * (z2 == 0), sA[av] * (z2 == 1)], axis=1)

    consts = {
        "G1": np.ascontiguousarray(G1, dtype=np.float32),
        "G2a": np.ascontiguousarray(G2a, dtype=np.float32),
        "G2b": np.ascontiguousarray(G2b, dtype=np.float32),
        "G3S": np.ascontiguousarray(G3S, dtype=np.float32),
        "R": np.ascontiguousarray(R, dtype=np.float32),
    }
    return consts, gm, sM, gz


def _host_finish(F, gm, sM, gz, b):
    # F: [B, 4, 8192]; n = t*128 + m'; pair t covers z' = t and z' = t + 64
    n = np.arange(8192)
    zp, mp = n // 128, n % 128
    w1 = gm[mp]
    w2 = sM[mp] * gz[zp]
    w3 = sM[mp] * gz[zp + 64]
    F = F.astype(np.float64)
    out = F[:, 0, :].sum(axis=1) + F[:, 1, :] @ w1 + F[:, 2, :] @ w2 + F[:, 3, :] @ w3
    return out + float(np.asarray(b).reshape(-1)[0])


# ------------------------------------------------------------- bass program
def build_bass():
    nc = bacc.Bacc("TRN2", target_bir_lowering=False)
    st = nc.declare_dram_parameter("state", [RPC, DIM], F32R, isOutput=False)
    g1 = nc.declare_dram_parameter("G1", [128, 256], F32R, isOutput=False)
    g2a = nc.declare_dram_parameter("G2a", [128, 256], F32R, isOutput=False)
    g2b = nc.declare_dram_parameter("G2b", [128, 256], F32R, isOutput=False)
    g3s = nc.declare_dram_parameter("G3S", [128, 384], F32R, isOutput=False)
    rw = nc.declare_dram_parameter("R", [128, 4], F32R, isOutput=False)
    fout = nc.declare_dram_parameter("F", [RPC, 4, 8192], F32, isOutput=True)
    dbg = nc.declare_dram_parameter("dbg", [1, 1], F32, isOutput=True)

    with ExitStack() as ctx:
        tc = ctx.enter_context(tile.TileContext(nc))
        const_pool = ctx.enter_context(tc.tile_pool(name="const", bufs=1))
        x0_pool = ctx.enter_context(tc.tile_pool(name="x0", bufs=2))
        y_pool = ctx.enter_context(tc.tile_pool(name="y", bufs=1))
        l2_pool = ctx.enter_context(tc.tile_pool(name="l2", bufs=3))
        sq_pool = ctx.enter_context(tc.tile_pool(name="sq", bufs=3))
        f_pool = ctx.enter_context(tc.tile_pool(name="f", bufs=3))
        psA_pool = ctx.enter_context(
            tc.tile_pool(name="psA", bufs=2, space=bass.MemorySpace.PSUM))
        psB_pool = ctx.enter_context(
            tc.tile_pool(name="psB", bufs=1, space=bass.MemorySpace.PSUM))
        ps4_pool = ctx.enter_context(
            tc.tile_pool(name="ps4", bufs=1, space=bass.MemorySpace.PSUM))
        scr_pool = ctx.enter_context(
            tc.tile_pool(name="scr", bufs=1, space=bass.MemorySpace.PSUM))

        G1 = const_pool.tile([128, 256], F32R)
        G2a = const_pool.tile([128, 256], F32R)
        G2b = const_pool.tile([128, 256], F32R)
        G3S = const_pool.tile([128, 384], F32R)
        R = const_pool.tile([128, 4], F32R)
        nc.sync.dma_start(G1[:], g1[:])
        nc.sync.dma_start(G2a[:], g2a[:])
        nc.sync.dma_start(G2b[:], g2b[:])
        nc.sync.dma_start(G3S[:], g3s[:])
        nc.sync.dma_start(R[:], rw[:])

        scratch = scr_pool.tile([128, 16], F32)

        # "touch" = tiny PE matmul whose only job is to make the PE observe
        # one producer-proc's semaphore, so real f32r matmuls (1 wait max)
        # never need multi-proc wait frontiers.
        def touch(ap_128xN):
            # N=2 moving cols: fp32r matmuls need even free dims (8B PSUM lines)
            return nc.tensor.matmul(
                scratch[0:1, 0:2], ap_128xN[:, 0:1], ap_128xN[:, 0:2],
                start=True, stop=True)

        def pin(after, *before):
            for binst in before:
                add_dep_helper(after.ins, binst.ins, False,
                               "touch ordering")

        tG1 = touch(G1[:])
        tG2a = touch(G2a[:])
        tG2b = touch(G2b[:])
        tG3S = touch(G3S[:])
        tR = touch(R[:])
        const_touches = [tG1, tG2a, tG2b, tG3S, tR]

        for r in range(RPC):
            # ---- load [z, (a, m)] in 4 chunk-tiles (16 a-values each)
            src = st[r].rearrange("(a m z) -> z a m", a=64, m=128, z=128)
            x0s, x0_touch = [], []
            for c in range(4):
                xt = x0_pool.tile([128, 2048], F32R, tag=f"x0{c}")
                xv = xt[:].rearrange("p (a m) -> p a m", a=16, m=128)
                nc.sync.dma_start(xv, src[:, 16 * c:16 * c + 16, :])
                x0s.append(xt)
                x0_touch.append(touch(xt[:]))

            yre = y_pool.tile([128, 8192], F32R, tag="yre")
            yim = y_pool.tile([128, 8192], F32R, tag="yim")

            # ---- P1: contract z.  out[m, z'_re|z'_im] per a.
            for g in range(16):
                ps = psA_pool.tile([128, 1024], F32, tag="ps")
                for q in range(4):
                    a = 4 * g + q
                    xa = x0s[a // 16][:, 128 * (a % 16):128 * (a % 16) + 128]
                    mm = nc.tensor.matmul(
                        ps[:, 256 * q:256 * q + 256], xa, G1[:],
                        start=True, stop=True)
                    if a % 16 == 0:
                        pin(mm, x0_touch[a // 16], *const_touches)
                pv = ps[:].rearrange("p (q t m) -> p q t m", q=4, t=2, m=128)
                yre_v = yre[:, 512 * g:512 * g + 512].rearrange(
                    "p (q m) -> p q m", q=4, m=128)
                yim_v = yim[:, 512 * g:512 * g + 512].rearrange(
                    "p (q m) -> p q m", q=4, m=128)
                eng = nc.vector if g % 2 == 0 else nc.scalar
                if g % 2 == 0:
                    eng.tensor_copy(yre_v, pv[:, :, 0, :])
                    eng.tensor_copy(yim_v, pv[:, :, 1, :])
                else:
                    eng.copy(yre_v, pv[:, :, 0, :])
                    eng.copy(yim_v, pv[:, :, 1, :])

            # Y col = a*128 + z' = (2a + h)*64 + t  (z' = h*64 + t).
            yre_v3 = yre[:].rearrange("p (k t) -> p k t", k=128, t=64)
            yim_v3 = yim[:].rearrange("p (k t) -> p k t", k=128, t=64)

            # Touch each engine's written Y regions (corner of every group
            # region) so P2's stationary reads need no multi-proc waits.
            y_touches = []
            for ten in (yre, yim):
                for par in (0, 1):
                    corner = ten[:].rearrange(
                        "p (gg rest) -> p gg rest", gg=16, rest=512)[:, par::2, 0]
                    y_touches.append(touch(corner))

            # ---- P2 (contract m) + P3 (G3) + SQ + P4, per z'-pair tile t
            for g in range(16):
                ps2 = psA_pool.tile([128, 1024], F32, tag="ps")
                l2re = l2_pool.tile([128, 512], F32R, tag="l2re")
                l2im = l2_pool.tile([128, 512], F32R, tag="l2im")
                for q in range(4):
                    zp = 4 * g + q
                    mm = nc.tensor.matmul(ps2[:, 256 * q:256 * q + 256],
                                          yre_v3[:, :, zp], G2a[:],
                                          start=True, stop=False)
                    if g == 0 and q == 0:
                        pin(mm, *y_touches)
                    nc.tensor.matmul(ps2[:, 256 * q:256 * q + 256],
                                     yim_v3[:, :, zp], G2b[:],
                                     start=False, stop=True)
                pv2 = ps2[:].rearrange("p (q t m) -> p q t m", q=4, t=2, m=128)
                l2re_v = l2re[:].rearrange("p (q m) -> p q m", q=4, m=128)
                l2im_v = l2im[:].rearrange("p (q m) -> p q m", q=4, m=128)
                if g % 2 == 0:
                    nc.vector.tensor_copy(l2re_v, pv2[:, :, 0, :])
                    nc.vector.tensor_copy(l2im_v, pv2[:, :, 1, :])
                else:
                    nc.scalar.copy(l2re_v, pv2[:, :, 0, :])
                    nc.scalar.copy(l2im_v, pv2[:, :, 1, :])

                ps3 = psB_pool.tile([128, 1024], F32, tag="ps3")
                nc.tensor.matmul(ps3[:, 0:512], G3S[:, 0:128], l2re[:],
                                 start=True, stop=False)
                nc.tensor.matmul(ps3[:, 0:512], G3S[:, 256:384], l2im[:],
                                 start=False, stop=True)
                nc.tensor.matmul(ps3[:, 512:1024], G3S[:, 128:256], l2re[:],
                                 start=True, stop=False)
                nc.tensor.matmul(ps3[:, 512:1024], G3S[:, 0:128], l2im[:],
                                 start=False, stop=True)

                sqre = sq_pool.tile([128, 512], F32R, tag="sqre")
                sqim = sq_pool.tile([128, 512], F32R, tag="sqim")
                nc.scalar.square(sqre[:], ps3[:, 0:512])
                nc.scalar.square(sqim[:], ps3[:, 512:1024])

                ps4 = ps4_pool.tile([4, 512], F32, tag="ps4")
                nc.tensor.matmul(ps4[:], R[:], sqre[:], start=True, stop=False)
                nc.tensor.matmul(ps4[:], R[:], sqim[:], start=False, stop=True)
                fsb = f_pool.tile([4, 512], F32, tag="fsb")
                nc.scalar.copy(fsb[:], ps4[:])
                nc.sync.dma_start(fout[r][:, 512 * g:512 * g + 512], fsb[:])

        # Keep touch outputs live: copy scratch corner out via DVE.
        dbg_sb = const_pool.tile([1, 1], F32)
        nc.vector.tensor_copy(dbg_sb[:], scratch[0:1, 0:1])
        nc.sync.dma_start(dbg[:], dbg_sb[:])
    nc.compile()
    return nc


# ------------------------------------------------------------------ wrapper
_CACHE = {}


def kernel(state, params, W, b):
    state = np.ascontiguousarray(np.asarray(state), dtype=np.float32)
    consts, gm, sM, gz = _host_precompute(np.asarray(params), np.asarray(W))

    if "nc" not in _CACHE:
        _CACHE["nc"] = build_bass()
    nc = _CACHE["nc"]

    in_maps = []
    for c in range(NCORES):
        m = {"state": state[RPC * c:RPC * (c + 1)]}
        m.update(consts)
        in_maps.append(m)
    res = run_bass_kernel_spmd(nc, in_maps, list(range(NCORES)))
    F = np.concatenate([res.results[c]["F"] for c in range(NCORES)], axis=0)
    out = _host_finish(F, gm, sM, gz, np.asarray(b))
    return out.astype(np.float32)


# revision 13
# speedup vs baseline: 6.9052x; 6.9052x over previous
"""Trainium2 Bass kernel for nn_AdvancedRegressionModel (20-qubit circuit regression).

Math: the reference circuit is out_b = sum_j g_j |(A psi_b)_j|^2 + b0 where
A = kron of 20 fused 2x2 gates (RY_k @ RX_k per wire) and g absorbs the CNOT
chain (a basis permutation), the <Z_i> measurements, and the linear head via
prefix-sign weights.  Split wires into blocks a(0-5), m(6-12), z(13-19);
g(a,m,z) = ga(a) + sA(a)*gm(m) + sA(a)*sM(m)*gz(z).

Device pipeline per batch row (4 rows/core, batch-sharded over 8 cores):
  load  [z:128, (a,m):8192]                                        (DMA)
  P1    contract z (stat-data matmuls, real -> complex)            (PE)
  P2    contract m (stat-data matmuls per z'-pair tile)            (PE)
  P3    apply kron(Ua, I2) on p=(a,z2) partitions (normal matmuls) (PE)
  SQ    squares of re/im (ScalarE, PSUM->SBUF)                     (ACT)
  P4    reduce-matmul with 4 sign-weight columns -> F[4, 8192]     (PE)
  host  finish: out_r = F0.1 + F1.gm + F2.sM*gz(t) + F3.sM*gz(t+64) + b

f32r matmuls carry at most ONE sync-wait, so cross-engine dependency
frontiers are funneled through tiny "touch" matmuls (one producer-proc each)
and PSUM drains alternate whole-group between DVE/ACT so slot-recycle
dependencies stay single-proc.
"""

import numpy as np
from contextlib import ExitStack

import concourse.mybir as mybir
from concourse import bacc, bass, tile
from concourse.bass_utils import run_bass_kernel_spmd
from concourse.tile_rust import add_dep_helper

NW = 20
DIM = 2 ** NW
BATCH = 32
NCORES = 8
RPC = BATCH // NCORES  # rows per core

F32 = mybir.dt.float32
F32R = mybir.dt.float32r


# ----------------------------------------------------------------- host math
def _gates(params):
    A = []
    for k in range(NW):
        c, s = np.cos(params[k] / 2), np.sin(params[k] / 2)
        RX = np.array([[c, -1j * s], [-1j * s, c]], dtype=np.complex128)
        c2, s2 = np.cos(params[k + NW] / 2), np.sin(params[k + NW] / 2)
        RY = np.array([[c2, -s2], [s2, c2]], dtype=np.complex128)
        A.append(RY @ RX)
    return A


def _kron_list(lst):
    out = lst[0]
    for x in lst[1:]:
        out = np.kron(out, x)
    return out


def _prefix_weights(Wv, wire_lo, wire_hi):
    n = wire_hi - wire_lo + 1
    v = np.arange(2 ** n)
    signs = np.stack([1 - 2 * ((v >> (n - 1 - k)) & 1) for k in range(n)])
    pref = np.cumprod(signs, axis=0)
    g = sum(Wv[wire_lo + i] * pref[i] for i in range(n))
    return g.astype(np.float64), pref[n - 1].astype(np.float64)


def _host_precompute(params, W):
    params = np.asarray(params, dtype=np.float64)
    Wv = np.asarray(W, dtype=np.float64).reshape(-1)
    A = _gates(params)
    Ua = _kron_list(A[0:6])        # 64x64
    Gq = np.kron(Ua, A[6])         # 128x128, contracts q = (a, m1)
    Uz = _kron_list(A[13:20])      # 128x128, contracts z
    Um2 = _kron_list(A[7:13])      # 64x64
    G3 = np.kron(Um2, np.eye(2))   # 128x128 on p2 = (m2, h)

    G1 = np.concatenate([Gq.real.T, Gq.imag.T], axis=1)
    G2a = np.concatenate([Uz.real.T, Uz.imag.T], axis=1)
    G2b = np.concatenate([-Uz.imag.T, Uz.real.T], axis=1)
    G3S = np.concatenate([G3.real.T, G3.imag.T, -G3.imag.T], axis=1)

    ga, sA = _prefix_weights(Wv, 0, 5)    # [64]
    gm, sM = _prefix_weights(Wv, 6, 12)   # [128]
    gz, _ = _prefix_weights(Wv, 13, 19)  # [128]

    # R' [128, 10]: partition p = m2'*2 + h
    p = np.arange(128)
    m2p, h = p >> 1, p & 1
    cols = [(h == 0).astype(float), (h == 1).astype(float)]
    for h0 in (0, 1):
        for m10 in (0, 1):
            cols.append((h == h0) * gm[m10 * 64 + m2p])
    for h0 in (0, 1):
        for m10 in (0, 1):
            cols.append((h == h0) * sM[m10 * 64 + m2p])
    R = np.stack(cols, axis=1)

    # free weights w[k, n], n = t*128 + z'; a' = h*32 + (t>>1), m1' = t&1
    n = np.arange(8192)
    t, zp = n >> 7, n & 127
    m1p = t & 1
    w = np.zeros((10, 8192))
    w[0] = ga[0 * 32 + (t >> 1)]
    w[1] = ga[1 * 32 + (t >> 1)]
    k = 2
    for h0 in (0, 1):
        for m10 in (0, 1):
            w[k] = (m1p == m10) * sA[h0 * 32 + (t >> 1)]
            k += 1
    for h0 in (0, 1):
        for m10 in (0, 1):
            w[k] = (m1p == m10) * sA[h0 * 32 + (t >> 1)] * gz[zp]
            k += 1

    consts = {
        "G1": np.ascontiguousarray(G1, dtype=np.float32),
        "G2a": np.ascontiguousarray(G2a, dtype=np.float32),
        "G2b": np.ascontiguousarray(G2b, dtype=np.float32),
        "G3S": np.ascontiguousarray(G3S, dtype=np.float32),
        "R": np.ascontiguousarray(R, dtype=np.float32),
    }
    return consts, w


def _host_finish(F, w, b):
    # F: [B, 10, 8192]
    out = np.einsum("bkn,kn->b", F.astype(np.float64), w)
    return out + float(np.asarray(b).reshape(-1)[0])


# ------------------------------------------------------------- bass program
def build_bass():
    nc = bacc.Bacc("TRN2", target_bir_lowering=False)
    st = nc.declare_dram_parameter("state", [RPC, DIM], F32R, isOutput=False)
    g1 = nc.declare_dram_parameter("G1", [128, 256], F32R, isOutput=False)
    g2a = nc.declare_dram_parameter("G2a", [128, 256], F32R, isOutput=False)
    g2b = nc.declare_dram_parameter("G2b", [128, 256], F32R, isOutput=False)
    g3s = nc.declare_dram_parameter("G3S", [128, 384], F32R, isOutput=False)
    rw = nc.declare_dram_parameter("R", [128, 10], F32R, isOutput=False)
    fout = nc.declare_dram_parameter("F", [RPC, 10, 8192], F32, isOutput=True)
    dbg = nc.declare_dram_parameter("dbg", [1, 1], F32, isOutput=True)

    with ExitStack() as ctx:
        tc = ctx.enter_context(tile.TileContext(nc))
        const_pool = ctx.enter_context(tc.tile_pool(name="const", bufs=1))
        x0_pool = ctx.enter_context(tc.tile_pool(name="x0", bufs=2))
        y_pool = ctx.enter_context(tc.tile_pool(name="y", bufs=1))
        l2_pool = ctx.enter_context(tc.tile_pool(name="l2", bufs=3))
        sq_pool = ctx.enter_context(tc.tile_pool(name="sq", bufs=3))
        f_pool = ctx.enter_context(tc.tile_pool(name="f", bufs=3))
        psA_pool = ctx.enter_context(
            tc.tile_pool(name="psA", bufs=2, space=bass.MemorySpace.PSUM))
        psB_pool = ctx.enter_context(
            tc.tile_pool(name="psB", bufs=1, space=bass.MemorySpace.PSUM))
        ps4_pool = ctx.enter_context(
            tc.tile_pool(name="ps4", bufs=1, space=bass.MemorySpace.PSUM))
        scr_pool = ctx.enter_context(
            tc.tile_pool(name="scr", bufs=1, space=bass.MemorySpace.PSUM))

        G1 = const_pool.tile([128, 256], F32R)
        G2a = const_pool.tile([128, 256], F32R)
        G2b = const_pool.tile([128, 256], F32R)
        G3S = const_pool.tile([128, 384], F32R)
        R = const_pool.tile([128, 10], F32R)
        nc.sync.dma_start(G1[:], g1[:])
        nc.sync.dma_start(G2a[:], g2a[:])
        nc.sync.dma_start(G2b[:], g2b[:])
        nc.sync.dma_start(G3S[:], g3s[:])
        nc.sync.dma_start(R[:], rw[:])

        scratch = scr_pool.tile([128, 16], F32)

        # "touch" = tiny PE matmul whose only job is to make the PE observe
        # one producer-proc's semaphore, so real f32r matmuls (1 wait max)
        # never need multi-proc wait frontiers.
        def touch(ap_128xN):
            # N=2 moving cols: fp32r matmuls need even free dims (8B PSUM lines)
            return nc.tensor.matmul(
                scratch[0:1, 0:2], ap_128xN[:, 0:1], ap_128xN[:, 0:2],
                start=True, stop=True)

        def pin(after, *before):
            for binst in before:
                add_dep_helper(after.ins, binst.ins, False,
                               "touch ordering")

        tG1 = touch(G1[:])
        tG2a = touch(G2a[:])
        tG2b = touch(G2b[:])
        tG3S = touch(G3S[:])
        tR = touch(R[:])
        const_touches = [tG1, tG2a, tG2b, tG3S, tR]

        for r in range(RPC):
            # ---- load [q:128, (m2:64, z:128)] -- plain contiguous reshape,
            # 8KB contiguous per partition per chunk DMA.
            srcv = st[r].rearrange("(q f) -> q f", q=128, f=8192)
            x0s, x0_touch = [], []
            for c in range(4):
                xt = x0_pool.tile([128, 2048], F32R, tag=f"x0{c}")
                nc.sync.dma_start(xt[:], srcv[:, 2048 * c:2048 * c + 2048])
                x0s.append(xt)
                x0_touch.append(touch(xt[:]))

            yre = y_pool.tile([128, 8192], F32R, tag="yre")
            yim = y_pool.tile([128, 8192], F32R, tag="yim")

            # ---- P0: contract q=(a,m1).  out[z, q'_re|q'_im] per m2.
            for g in range(16):
                ps = psA_pool.tile([128, 1024], F32, tag="ps")
                for q in range(4):
                    a = 4 * g + q
                    xa = x0s[a // 16][:, 128 * (a % 16):128 * (a % 16) + 128]
                    mm = nc.tensor.matmul(
                        ps[:, 256 * q:256 * q + 256], xa, G1[:],
                        start=True, stop=True)
                    if a % 16 == 0:
                        pin(mm, x0_touch[a // 16], *const_touches)
                pv = ps[:].rearrange("p (q t m) -> p q t m", q=4, t=2, m=128)
                yre_v = yre[:, 512 * g:512 * g + 512].rearrange(
                    "p (q m) -> p q m", q=4, m=128)
                yim_v = yim[:, 512 * g:512 * g + 512].rearrange(
                    "p (q m) -> p q m", q=4, m=128)
                eng = nc.vector if g % 2 == 0 else nc.scalar
                if g % 2 == 0:
                    eng.tensor_copy(yre_v, pv[:, :, 0, :])
                    eng.tensor_copy(yim_v, pv[:, :, 1, :])
                else:
                    eng.copy(yre_v, pv[:, :, 0, :])
                    eng.copy(yim_v, pv[:, :, 1, :])

            # Y col = a*128 + z' = (2a + h)*64 + t  (z' = h*64 + t).
            yre_v3 = yre[:].rearrange("p (k t) -> p k t", k=128, t=64)
            yim_v3 = yim[:].rearrange("p (k t) -> p k t", k=128, t=64)

            # Touch each engine's written Y regions (corner of every group
            # region) so P2's stationary reads need no multi-proc waits.
            y_touches = []
            for ten in (yre, yim):
                for par in (0, 1):
                    corner = ten[:].rearrange(
                        "p (gg rest) -> p gg rest", gg=16, rest=512)[:, par::2, 0]
                    y_touches.append(touch(corner))

            # ---- P2 (contract m) + P3 (G3) + SQ + P4, per z'-pair tile t
            for g in range(16):
                ps2 = psA_pool.tile([128, 1024], F32, tag="ps")
                l2re = l2_pool.tile([128, 512], F32R, tag="l2re")
                l2im = l2_pool.tile([128, 512], F32R, tag="l2im")
                for q in range(4):
                    zp = 4 * g + q
                    mm = nc.tensor.matmul(ps2[:, 256 * q:256 * q + 256],
                                          yre_v3[:, :, zp], G2a[:],
                                          start=True, stop=False)
                    if g == 0 and q == 0:
                        pin(mm, *y_touches)
                    nc.tensor.matmul(ps2[:, 256 * q:256 * q + 256],
                                     yim_v3[:, :, zp], G2b[:],
                                     start=False, stop=True)
                pv2 = ps2[:].rearrange("p (q t m) -> p q t m", q=4, t=2, m=128)
                l2re_v = l2re[:].rearrange("p (q m) -> p q m", q=4, m=128)
                l2im_v = l2im[:].rearrange("p (q m) -> p q m", q=4, m=128)
                if g % 2 == 0:
                    nc.vector.tensor_copy(l2re_v, pv2[:, :, 0, :])
                    nc.vector.tensor_copy(l2im_v, pv2[:, :, 1, :])
                else:
                    nc.scalar.copy(l2re_v, pv2[:, :, 0, :])
                    nc.scalar.copy(l2im_v, pv2[:, :, 1, :])

                ps3 = psB_pool.tile([128, 1024], F32, tag="ps3")
                nc.tensor.matmul(ps3[:, 0:512], G3S[:, 0:128], l2re[:],
                                 start=True, stop=False)
                nc.tensor.matmul(ps3[:, 0:512], G3S[:, 256:384], l2im[:],
                                 start=False, stop=True)
                nc.tensor.matmul(ps3[:, 512:1024], G3S[:, 128:256], l2re[:],
                                 start=True, stop=False)
                nc.tensor.matmul(ps3[:, 512:1024], G3S[:, 0:128], l2im[:],
                                 start=False, stop=True)

                sqre = sq_pool.tile([128, 512], F32R, tag="sqre")
                sqim = sq_pool.tile([128, 512], F32R, tag="sqim")
                nc.scalar.square(sqre[:], ps3[:, 0:512])
                nc.scalar.square(sqim[:], ps3[:, 512:1024])

                ps4 = ps4_pool.tile([10, 512], F32, tag="ps4")
                nc.tensor.matmul(ps4[:], R[:], sqre[:], start=True, stop=False)
                nc.tensor.matmul(ps4[:], R[:], sqim[:], start=False, stop=True)
                fsb = f_pool.tile([10, 512], F32, tag="fsb")
                nc.scalar.copy(fsb[:], ps4[:])
                nc.sync.dma_start(fout[r][:, 512 * g:512 * g + 512], fsb[:])

        # Keep touch outputs live: copy scratch corner out via DVE.
        dbg_sb = const_pool.tile([1, 1], F32)
        nc.vector.tensor_copy(dbg_sb[:], scratch[0:1, 0:1])
        nc.sync.dma_start(dbg[:], dbg_sb[:])
    nc.compile()
    return nc


# ------------------------------------------------------------------ wrapper
_CACHE = {}


def kernel(state, params, W, b):
    state = np.ascontiguousarray(np.asarray(state), dtype=np.float32)
    consts, w = _host_precompute(np.asarray(params), np.asarray(W))

    if "nc" not in _CACHE:
        _CACHE["nc"] = build_bass()
    nc = _CACHE["nc"]

    in_maps = []
    for c in range(NCORES):
        m = {"state": state[RPC * c:RPC * (c + 1)]}
        m.update(consts)
        in_maps.append(m)
    res = run_bass_kernel_spmd(nc, in_maps, list(range(NCORES)))
    F = np.concatenate([res.results[c]["F"] for c in range(NCORES)], axis=0)
    out = _host_finish(F, w, np.asarray(b))
    return out.astype(np.float32)


# revision 17
# speedup vs baseline: 8.4309x; 1.2210x over previous
"""Trainium2 Bass kernel for nn_AdvancedRegressionModel (20-qubit circuit regression).

Math: the reference circuit collapses to out_b = sum_j g_j |(A psi_b)_j|^2 + b0
where A = kron of 20 fused 2x2 gates (RY_k @ RX_k per wire) and g absorbs the
CNOT chain (a basis permutation), the <Z_i> measurements, and the linear head
via prefix-sign weights: g(a,m,z) = ga(a) + sA(a)*gm(m) + sA(a)*sM(m)*gz(z).

Wire blocks: q = wires 0-6 (a:0-5 + m1:6), m2 = wires 7-12, z = wires 13-19.
Per batch row (4 rows/core, batch-sharded over 8 cores):
  load  [q:128, (m2:64, z:128)] - plain contiguous chunks          (DMA)
  P0    contract q: stat-data matmuls per m2 tile -> [z, q'_re|im] (PE)
  P1'   contract z: stat-data matmuls per stride-64 column pick t
        -> partitions p2 = 2*m2 + h  (h = top bit of q')           (PE)
  P2'   apply kron(Um2, I2) on p2 (normal matmuls, f32r)           (PE)
  SQ    squares of re/im PSUM -> SBUF                              (ACT)
  P4    reduce-matmul with 16-col sign-weight stationary R
        -> F[10, (t:64, z':128)] per row                           (PE)
  host  finish: out_r = sum_kn F[k,n] * w_k(n) + b

f32r (tf32-like) keeps rel err ~2.6e-3.  Self-loading f32r matmuls allow only
ONE sync-wait, so cross-engine dependency frontiers are funneled through tiny
"touch" matmuls; PSUM drains split DVE/ACT ~2:1 by group for engine balance.
"""

import numpy as np
from contextlib import ExitStack

import concourse.mybir as mybir
from concourse import bacc, bass, tile
from concourse.bass_utils import run_bass_kernel_spmd
from concourse.tile_rust import add_dep_helper

NW = 20
DIM = 2 ** NW
BATCH = 32
NCORES = 8
RPC = BATCH // NCORES  # rows per core

F32 = mybir.dt.float32
F32R = mybir.dt.float32r
BF16 = mybir.dt.bfloat16
import os
USE_BF16 = os.environ.get("KBF16", "0") == "1"
DT = BF16 if USE_BF16 else F32R


# ----------------------------------------------------------------- host math
def _gates(params):
    A = []
    for k in range(NW):
        c, s = np.cos(params[k] / 2), np.sin(params[k] / 2)
        RX = np.array([[c, -1j * s], [-1j * s, c]], dtype=np.complex128)
        c2, s2 = np.cos(params[k + NW] / 2), np.sin(params[k + NW] / 2)
        RY = np.array([[c2, -s2], [s2, c2]], dtype=np.complex128)
        A.append(RY @ RX)
    return A


def _kron_list(lst):
    out = lst[0]
    for x in lst[1:]:
        out = np.kron(out, x)
    return out


def _prefix_weights(Wv, wire_lo, wire_hi):
    n = wire_hi - wire_lo + 1
    v = np.arange(2 ** n)
    signs = np.stack([1 - 2 * ((v >> (n - 1 - k)) & 1) for k in range(n)])
    pref = np.cumprod(signs, axis=0)
    g = sum(Wv[wire_lo + i] * pref[i] for i in range(n))
    return g.astype(np.float64), pref[n - 1].astype(np.float64)


def _host_precompute(params, W):
    params = np.asarray(params, dtype=np.float64)
    Wv = np.asarray(W, dtype=np.float64).reshape(-1)
    A = _gates(params)
    Ua = _kron_list(A[0:6])        # 64x64
    Gq = np.kron(Ua, A[6])         # 128x128, contracts q = (a, m1)
    Uz = _kron_list(A[13:20])      # 128x128, contracts z
    Um2 = _kron_list(A[7:13])      # 64x64
    G3 = np.kron(Um2, np.eye(2))   # 128x128 on p2 = (m2, h)

    G1 = np.concatenate([Gq.real.T, Gq.imag.T], axis=1)
    G2a = np.concatenate([Uz.real.T, Uz.imag.T], axis=1)
    G2b = np.concatenate([-Uz.imag.T, Uz.real.T], axis=1)
    G3S = np.concatenate([G3.real.T, G3.imag.T, -G3.imag.T], axis=1)

    ga, sA = _prefix_weights(Wv, 0, 5)    # [64]
    gm, sM = _prefix_weights(Wv, 6, 12)   # [128]
    gz, _ = _prefix_weights(Wv, 13, 19)  # [128]

    # R' [128, 10]: partition p = m2'*2 + h
    p = np.arange(128)
    m2p, h = p >> 1, p & 1
    cols = [(h == 0).astype(float), (h == 1).astype(float)]
    for h0 in (0, 1):
        for m10 in (0, 1):
            cols.append((h == h0) * gm[m10 * 64 + m2p])
    for h0 in (0, 1):
        for m10 in (0, 1):
            cols.append((h == h0) * sM[m10 * 64 + m2p])
    R = np.stack(cols, axis=1)
    R = np.concatenate([R, np.zeros((128, 6))], axis=1)  # pad to 16 cols

    # free weights w[k, n], n = t*128 + z'; a' = h*32 + (t>>1), m1' = t&1
    n = np.arange(8192)
    t, zp = n >> 7, n & 127
    m1p = t & 1
    w = np.zeros((10, 8192))
    w[0] = ga[0 * 32 + (t >> 1)]
    w[1] = ga[1 * 32 + (t >> 1)]
    k = 2
    for h0 in (0, 1):
        for m10 in (0, 1):
            w[k] = (m1p == m10) * sA[h0 * 32 + (t >> 1)]
            k += 1
    for h0 in (0, 1):
        for m10 in (0, 1):
            w[k] = (m1p == m10) * sA[h0 * 32 + (t >> 1)] * gz[zp]
            k += 1

    import ml_dtypes
    cdt = ml_dtypes.bfloat16 if USE_BF16 else np.float32
    consts = {
        "G1": np.ascontiguousarray(G1, dtype=cdt),
        "G2a": np.ascontiguousarray(G2a, dtype=cdt),
        "G2b": np.ascontiguousarray(G2b, dtype=cdt),
        "G3S": np.ascontiguousarray(G3S, dtype=cdt),
        "R": np.ascontiguousarray(R, dtype=np.float32),
    }
    return consts, w


def _host_finish(F, w, b):
    # F: [B, 10, 8192]
    out = np.einsum("bkn,kn->b", F.astype(np.float64), w)
    return out + float(np.asarray(b).reshape(-1)[0])


# ------------------------------------------------------------- bass program
def build_bass():
    nc = bacc.Bacc("TRN2", target_bir_lowering=False)
    st = nc.declare_dram_parameter("state", [RPC, DIM], DT, isOutput=False)
    g1 = nc.declare_dram_parameter("G1", [128, 256], DT, isOutput=False)
    g2a = nc.declare_dram_parameter("G2a", [128, 256], DT, isOutput=False)
    g2b = nc.declare_dram_parameter("G2b", [128, 256], DT, isOutput=False)
    g3s = nc.declare_dram_parameter("G3S", [128, 384], DT, isOutput=False)
    rw = nc.declare_dram_parameter("R", [128, 16], F32R, isOutput=False)
    fout = nc.declare_dram_parameter("F", [RPC, 10, 8192], F32, isOutput=True)
    dbg = nc.declare_dram_parameter("dbg", [1, 1], F32, isOutput=True)

    with ExitStack() as ctx:
        tc = ctx.enter_context(tile.TileContext(nc))
        const_pool = ctx.enter_context(tc.tile_pool(name="const", bufs=1))
        x0_pool = ctx.enter_context(tc.tile_pool(name="x0", bufs=2))
        y_pool = ctx.enter_context(tc.tile_pool(name="y", bufs=1))
        l2_pool = ctx.enter_context(tc.tile_pool(name="l2", bufs=3))
        sq_pool = ctx.enter_context(tc.tile_pool(name="sq", bufs=3))
        f_pool = ctx.enter_context(tc.tile_pool(name="f", bufs=3))
        psA_pool = ctx.enter_context(
            tc.tile_pool(name="psA", bufs=2, space=bass.MemorySpace.PSUM))
        psB_pool = ctx.enter_context(
            tc.tile_pool(name="psB", bufs=1, space=bass.MemorySpace.PSUM))
        ps4_pool = ctx.enter_context(
            tc.tile_pool(name="ps4", bufs=1, space=bass.MemorySpace.PSUM))
        scr_pool = ctx.enter_context(
            tc.tile_pool(name="scr", bufs=1, space=bass.MemorySpace.PSUM))

        G1 = const_pool.tile([128, 256], DT)
        G2a = const_pool.tile([128, 256], DT)
        G2b = const_pool.tile([128, 256], DT)
        G3S = const_pool.tile([128, 384], DT)
        R = const_pool.tile([128, 16], F32R)
        scratch = scr_pool.tile([128, 16], F32)

        # "touch" = tiny PE matmul whose only job is to make the PE observe
        # one producer-proc's semaphore, so real f32r matmuls (1 wait max)
        # never need multi-proc wait frontiers.
        def touch(ap_128xN):
            # N=2 moving cols: fp32r matmuls need even free dims (8B PSUM lines)
            return nc.tensor.matmul(
                scratch[0:1, 0:2], ap_128xN[:, 0:1], ap_128xN[:, 0:2],
                start=True, stop=True)

        def pin(after, *before):
            for binst in before:
                add_dep_helper(after.ins, binst.ins, False,
                               "touch ordering")

        def load_row(r):
            srcv = st[r].rearrange("(q f) -> q f", q=128, f=8192)
            x0s, x0t = [], []
            for c in range(16):
                xt = x0_pool.tile([128, 512], DT, tag=f"x0{c}")
                nc.sync.dma_start(xt[:], srcv[:, 512 * c:512 * c + 512])
                x0s.append(xt)
                x0t.append(touch(xt[:]))
            return x0s, x0t

        # row 0's 16 chunk DMAs go out first so they own all queues at start
        preload = {0: load_row(0)}

        nc.sync.dma_start(G1[:], g1[:])
        nc.sync.dma_start(G2a[:], g2a[:])
        nc.sync.dma_start(G2b[:], g2b[:])
        nc.sync.dma_start(G3S[:], g3s[:])
        nc.sync.dma_start(R[:], rw[:])
        tG1 = touch(G1[:])
        tG2a = touch(G2a[:])
        tG2b = touch(G2b[:])
        tG3S = touch(G3S[:])
        tR = touch(R[:])
        const_touches = [tG1, tG2a, tG2b, tG3S, tR]

        for r in range(RPC):
            # ---- load [q:128, (m2:64, z:128)] in 16 contiguous chunk DMAs
            x0s, x0_touch = preload[r] if r in preload else load_row(r)

            yre = y_pool.tile([128, 8192], DT, tag="yre")
            yim = y_pool.tile([128, 8192], DT, tag="yim")

            # ---- P0: contract q=(a,m1).  out[z, q'_re|q'_im] per m2.
            for g in range(16):
                ps = psA_pool.tile([128, 1024], F32, tag="ps")
                for q in range(4):
                    a = 4 * g + q
                    xa = x0s[a // 4][:, 128 * (a % 4):128 * (a % 4) + 128]
                    mm = nc.tensor.matmul(
                        ps[:, 256 * q:256 * q + 256], xa, G1[:],
                        start=True, stop=True)
                    if a % 4 == 0:
                        pin(mm, x0_touch[a // 4], *const_touches)
                pv = ps[:].rearrange("p (q t m) -> p q t m", q=4, t=2, m=128)
                yre_v = yre[:, 512 * g:512 * g + 512].rearrange(
                    "p (q m) -> p q m", q=4, m=128)
                yim_v = yim[:, 512 * g:512 * g + 512].rearrange(
                    "p (q m) -> p q m", q=4, m=128)
                if g % 3 == 2:
                    nc.scalar.copy(yre_v, pv[:, :, 0, :])
                    nc.scalar.copy(yim_v, pv[:, :, 1, :])
                else:
                    nc.vector.tensor_copy(yre_v, pv[:, :, 0, :])
                    nc.vector.tensor_copy(yim_v, pv[:, :, 1, :])

            # Y col = a*128 + z' = (2a + h)*64 + t  (z' = h*64 + t).
            yre_v3 = yre[:].rearrange("p (k t) -> p k t", k=128, t=64)
            yim_v3 = yim[:].rearrange("p (k t) -> p k t", k=128, t=64)

            # Touch each engine's written Y regions (corner of every group
            # region) so P2's stationary reads need no multi-proc waits.
            y_touches = []
            for ten in (yre, yim):
                corner = ten[:].rearrange(
                    "p (gg rest) -> p gg rest", gg=16, rest=512)[:, :, 0]
                y_touches.append(touch(corner))

            # ---- P2 (contract m) + P3 (G3) + SQ + P4, per z'-pair tile t
            for g in range(16):
                ps2 = psA_pool.tile([128, 1024], F32, tag="ps")
                l2re = l2_pool.tile([128, 512], DT, tag="l2re")
                l2im = l2_pool.tile([128, 512], DT, tag="l2im")
                for q in range(4):
                    zp = 4 * g + q
                    mm = nc.tensor.matmul(ps2[:, 256 * q:256 * q + 256],
                                          yre_v3[:, :, zp], G2a[:],
                                          start=True, stop=False)
                    if g == 0 and q == 0:
                        pin(mm, *y_touches)
                    nc.tensor.matmul(ps2[:, 256 * q:256 * q + 256],
                                     yim_v3[:, :, zp], G2b[:],
                                     start=False, stop=True)
                pv2 = ps2[:].rearrange("p (q t m) -> p q t m", q=4, t=2, m=128)
                l2re_v = l2re[:].rearrange("p (q m) -> p q m", q=4, m=128)
                l2im_v = l2im[:].rearrange("p (q m) -> p q m", q=4, m=128)
                if g % 3 == 2:
                    nc.scalar.copy(l2re_v, pv2[:, :, 0, :])
                    nc.scalar.copy(l2im_v, pv2[:, :, 1, :])
                else:
                    nc.vector.tensor_copy(l2re_v, pv2[:, :, 0, :])
                    nc.vector.tensor_copy(l2im_v, pv2[:, :, 1, :])

                ps3 = psB_pool.tile([128, 1024], F32, tag="ps3")
                nc.tensor.matmul(ps3[:, 0:512], G3S[:, 0:128], l2re[:],
                                 start=True, stop=False)
                nc.tensor.matmul(ps3[:, 0:512], G3S[:, 256:384], l2im[:],
                                 start=False, stop=True)
                nc.tensor.matmul(ps3[:, 512:1024], G3S[:, 128:256], l2re[:],
                                 start=True, stop=False)
                nc.tensor.matmul(ps3[:, 512:1024], G3S[:, 0:128], l2im[:],
                                 start=False, stop=True)

                sq = sq_pool.tile([128, 1024], F32R, tag="sq")
                nc.scalar.square(sq[:], ps3[:])

                ps4 = ps4_pool.tile([16, 512], F32, tag="ps4")
                nc.tensor.matmul(ps4[:], R[:], sq[:, 0:512], start=True, stop=False)
                nc.tensor.matmul(ps4[:], R[:], sq[:, 512:1024], start=False, stop=True)
                fsb = f_pool.tile([10, 512], F32, tag="fsb")
                nc.vector.tensor_copy(fsb[:], ps4[0:10, :])
                nc.sync.dma_start(fout[r][:, 512 * g:512 * g + 512], fsb[:])

        # Keep touch outputs live: copy scratch corner out via DVE.
        dbg_sb = const_pool.tile([1, 1], F32)
        nc.vector.tensor_copy(dbg_sb[:], scratch[0:1, 0:1])
        nc.sync.dma_start(dbg[:], dbg_sb[:])
    nc.compile()
    return nc


# ------------------------------------------------------------------ wrapper
_CACHE = {}


def kernel(state, params, W, b):
    if USE_BF16:
        import ml_dtypes
        state = np.ascontiguousarray(np.asarray(state), dtype=ml_dtypes.bfloat16)
    else:
        state = np.ascontiguousarray(np.asarray(state), dtype=np.float32)
    consts, w = _host_precompute(np.asarray(params), np.asarray(W))

    if "nc" not in _CACHE:
        _CACHE["nc"] = build_bass()
    nc = _CACHE["nc"]

    in_maps = []
    for c in range(NCORES):
        m = {"state": state[RPC * c:RPC * (c + 1)]}
        m.update(consts)
        in_maps.append(m)
    res = run_bass_kernel_spmd(nc, in_maps, list(range(NCORES)))
    F = np.concatenate([res.results[c]["F"] for c in range(NCORES)], axis=0)
    out = _host_finish(F, w, np.asarray(b))
    return out.astype(np.float32)
